# revision 6
# baseline (speedup 1.0000x reference)
"""Multi-head causal attention on 8 Trainium2 NeuronCores.

Problem: B=2, T=2048, C=1024, H=16, HS=64 (fp32), causal mask.

Sharding: 8 cores = 2 batches x 4 head-groups (4 heads each). Each core
computes q/k/v projections + attention + its partial output projection for
its 4 heads of its batch; the host sums the 4 per-batch partials (the
all-reduce of the tensor-parallel output projection) and adds the bias.

v3 design:
  * all matmul operands fp16 (1 cyc/row on PE, half the DMA/SBUF bytes);
    accumulation stays fp32 in PSUM so error ~1e-3 << 2e-2 gate.
  * scores row-tiled: per head-pair, head A contracts over PE rows 0-63,
    head B over rows 64-127 (tile_position via base_partition) -> both
    heads' score tiles stream in one 512-cycle window.
  * DMA is descriptor-rate bound (~23ns/desc/queue): every transfer uses
    4KB-contiguous partition lines via host-side prepacking (weights and
    x in SBUF layout; y out in a packed [128, 8, 2048] layout that the
    host unpacks), spread over all 4 engine queues.
  * scalar engine: exp + denominator sums copies only; normalize chain is
    spread scalar->gpsimd->vector->gpsimd so no in-order queue serializes
    a block boundary; AV runs 2 steps behind scores (pend deque) so the
    single-buffered pa bank has time to drain.
"""

import numpy as np

B, T, C, H, HS = 2, 2048, 1024, 16, 64
NCORES = 8
HPC = 4            # heads per core
NKC = C // 128     # contraction chunks (8)
NJ = T // 512      # tq chunks (4)
NTS = T // 128     # ts chunks (16)

_NC_CACHE = {}


def _build_nc():
    if "nc" in _NC_CACHE:
        return _NC_CACHE["nc"]
    from contextlib import ExitStack
    import concourse.bass as bass
    from concourse import bacc, tile, mybir

    f32 = mybir.dt.float32
    f16 = mybir.dt.float16
    EXP = mybir.ActivationFunctionType.Exp

    nc = bacc.Bacc("TRN2", target_bir_lowering=False, debug=False,
                   enable_asserts=False, num_devices=NCORES)

    # prepacked inputs: per-partition-contiguous 4KB lines
    xt_d = nc.dram_tensor("xt_p", (128, NKC, T), f16, kind="ExternalInput").ap()
    wq_d = nc.dram_tensor("wq_p", (128, NKC * 256), f16, kind="ExternalInput").ap()
    wk_d = nc.dram_tensor("wk_p", (128, NKC * 256), f16, kind="ExternalInput").ap()
    wv_d = nc.dram_tensor("wv_p", (128, NKC * 256), f16, kind="ExternalInput").ap()
    wp_d = nc.dram_tensor("wp_p", (128, 2 * C), f16, kind="ExternalInput").ap()
    # packed output: [p, mp, 1024*h + c] = y[128*(2*mp+h) + p, c]
    y_d = nc.dram_tensor("y", (128, 8, 2048), f16, kind="ExternalOutput").ap()

    scale = float(1.0 / np.sqrt(HS))

    with tile.TileContext(nc) as tc, ExitStack() as ctx:
        persist = ctx.enter_context(tc.tile_pool(name="persist", bufs=1))
        work = ctx.enter_context(tc.tile_pool(name="work", bufs=2))
        small = ctx.enter_context(tc.tile_pool(name="small", bufs=2))
        outp = ctx.enter_context(tc.tile_pool(name="outp", bufs=2))
        psS = ctx.enter_context(tc.tile_pool(name="psS", bufs=1, space="PSUM"))
        psatt = ctx.enter_context(tc.tile_pool(name="psatt", bufs=1, space="PSUM"))
        psaux = ctx.enter_context(tc.tile_pool(name="psaux", bufs=2, space="PSUM"))

        # ---- persistent SBUF tensors (fp16 matmul operands) ----
        xt = [persist.tile([128, T], f16, tag=f"xt{c}", name=f"xt{c}") for c in range(NKC)]
        wq_sb = persist.tile([128, NKC, 256], f16, tag="wq")
        wk_sb = persist.tile([128, NKC, 256], f16, tag="wk")
        wv_sb = persist.tile([128, NKC, 256], f16, tag="wv")
        wp_sb = persist.tile([128, 2, C], f16, tag="wp")
        qT = [persist.tile([128, T], f16, tag=f"qT{p}", name=f"qT{p}") for p in range(2)]
        kT = [persist.tile([128, T], f16, tag=f"kT{p}", name=f"kT{p}") for p in range(2)]
        # v: per ts-chunk t and head h, [:, t, h, 0:64] = v dims, [..., 64] = 1.0
        vt = persist.tile([128, NTS, HPC, 65], f16, tag="vt")
        attnT = [persist.tile([128, T], f16, tag=f"attnT{p}", name=f"attnT{p}") for p in range(2)]

        # ---- loads: 4KB lines, consumption order, all 4 engine queues ----
        wqf = wq_sb.rearrange("p k n -> p (k n)")
        wkf = wk_sb.rearrange("p k n -> p (k n)")
        wvf = wv_sb.rearrange("p k n -> p (k n)")
        wpf = wp_sb.rearrange("p k n -> p (k n)")
        nc.scalar.dma_start(out=wqf, in_=wq_d)
        nc.sync.dma_start(out=xt[0], in_=xt_d[:, 0, :])
        nc.gpsimd.dma_start(out=xt[1], in_=xt_d[:, 1, :])
        nc.sync.dma_start(out=xt[2], in_=xt_d[:, 2, :])
        nc.scalar.dma_start(out=xt[3], in_=xt_d[:, 3, :])
        nc.gpsimd.dma_start(out=xt[4], in_=xt_d[:, 4, :])
        nc.sync.dma_start(out=xt[5], in_=xt_d[:, 5, :])
        nc.scalar.dma_start(out=xt[6], in_=xt_d[:, 6, :])
        nc.gpsimd.dma_start(out=xt[7], in_=xt_d[:, 7, :])
        nc.gpsimd.dma_start(out=wvf, in_=wv_d)
        nc.scalar.dma_start(out=wkf, in_=wk_d)
        nc.sync.dma_start(out=wpf, in_=wp_d)

        # exp table preload (after the load triggers so they fire first)
        dummy = small.tile([1, 2], f32, tag="dummy")
        nc.vector.memset(dummy, 0.0)
        nc.scalar.activation(out=dummy[0:1, 1:2], in_=dummy[0:1, 0:1], func=EXP)

        nc.vector.memset(vt[:, :, :, 64:65], 1.0)

        # ---------- emission helpers ----------
        filler = []     # queue of closures emitting independent PE work

        def pull(n):
            for _ in range(n):
                if filler:
                    filler.pop(0)()

        def qk_chain_units(pair, dst, w_sb, J, name):
            # split one 8-matmul accumulation chain into 4 filler units
            ps = psaux.tile([128, 512], f32, tag="aux", name=name)

            def unit(c0):
                def f():
                    for c in (c0, c0 + 1):
                        nc.tensor.matmul(
                            ps,
                            lhsT=w_sb[:, c, 128 * pair:128 * pair + 128],
                            rhs=xt[c][:, 512 * J:512 * J + 512],
                            start=(c == 0), stop=(c == NKC - 1))
                    if c0 == NKC - 2:
                        nc.vector.tensor_copy(
                            out=dst[:, 512 * J:512 * J + 512], in_=ps)
                return f
            return [unit(c0) for c0 in range(0, NKC, 2)]

        def qk_chain(pair, dst, w_sb, J, name):
            for u in qk_chain_units(pair, dst, w_sb, J, name):
                u()

        def v_chain(t):
            ps = psaux.tile([128, 512], f32, tag="aux", name=f"v_{t}")
            for c in range(NKC):
                nc.tensor.matmul(
                    ps[:, 0:256],
                    lhsT=xt[c][:, 128 * t:128 * t + 128],
                    rhs=wv_sb[:, c, :],
                    start=(c == 0), stop=(c == NKC - 1))
            nc.vector.tensor_copy(out=vt[:, t, :, 0:64], in_=ps[:, 0:256])

        yo2 = {}

        def proj_tile(m, n):
            # proj for row-block m, col-half n; packed into [128,2048] pair
            # tiles (mp = m//2) that DMA out with 4KB partition lines.
            mp, h = divmod(m, 2)
            py_ = psaux.tile([128, 512], f32, tag="aux", name=f"y_{m}_{n}")
            for pair in range(2):
                nc.tensor.matmul(
                    py_,
                    lhsT=attnT[pair][:, 128 * m:128 * m + 128],
                    rhs=wp_sb[:, pair, 512 * n:512 * n + 512],
                    start=(pair == 0), stop=(pair == 1))
            if mp not in yo2:
                yo2[mp] = outp.tile([128, 2048], f16, tag=f"yo{mp % 2}",
                                    name=f"yo2_{mp}")
            nc.vector.tensor_copy(
                out=yo2[mp][:, 1024 * h + 512 * n:1024 * h + 512 * n + 512],
                in_=py_)
            if h == 1 and n == 1:
                eng = nc.sync if mp % 2 == 0 else nc.gpsimd
                eng.dma_start(out=y_d[:, mp, :], in_=yo2.pop(mp))

        def att_pair_block(pair, J, extra=2):
            """Attention for both heads of `pair` over tq block J.

            Scores for heads A (=2*pair) and B run row-tiled (concurrent on
            HW); AV lags scores by 2 steps so the single-buffered pa banks
            have time to drain through the previous block's normalize.
            """
            nch = 4 * J + 4
            jsl = slice(512 * J, 512 * J + 512)
            pa = [psatt.tile([65, 512], f32, tag=f"pa{hh}", name=f"pa{hh}_{pair}_{J}")
                  for hh in range(2)]
            pend = []          # AV two steps behind scores

            def flush_av(last=False):
                etA, etB, t0, t1 = pend.pop(0)
                for half, t in ((0, t0), (1, t1)):
                    for hh, et in ((0, etA), (1, etB)):
                        nc.tensor.matmul(
                            pa[hh], lhsT=vt[:, t, 2 * pair + hh, :],
                            rhs=et[:, 512 * half:512 * half + 512],
                            start=(t == 0),
                            stop=(last and t == nch - 1))

            for u in range(nch // 2):
                t0, t1 = 2 * u, 2 * u + 1
                ss = [psS.tile([128, 1024], f32, tag=f"s{hh}", name=f"ss{hh}_{pair}_{J}_{u}")
                      for hh in range(2)]
                for half, t in ((0, t0), (1, t1)):
                    for hh in range(2):
                        hsl = slice(64 * hh, 64 * hh + 64)
                        nc.tensor.matmul(
                            ss[hh][:, 512 * half:512 * half + 512],
                            lhsT=kT[pair][hsl, 128 * t:128 * t + 128],
                            rhs=qT[pair][hsl, jsl],
                            start=True, stop=True)
                et = [work.tile([128, 1024], f16, tag=f"et{hh}", bufs=3,
                                name=f"et{hh}_{pair}_{J}_{u}")
                      for hh in range(2)]
                for hh in range(2):
                    nc.scalar.activation(out=et[hh], in_=ss[hh], func=EXP, scale=scale)
                for half, t in ((0, t0), (1, t1)):
                    if t >= 4 * J:
                        d = t - 4 * J
                        for hh in range(2):
                            sl = et[hh][:, 512 * half:512 * half + 512]
                            # keep el iff f >= p + 128*d
                            nc.gpsimd.affine_select(
                                out=sl, in_=sl,
                                compare_op=mybir.AluOpType.is_ge,
                                fill=0.0, base=-128 * d,
                                pattern=[[1, 512]], channel_multiplier=-1)
                pend.append((et[0], et[1], t0, t1))
                if len(pend) > 2:
                    flush_av()
                pull(extra)
            while pend:
                flush_av(last=len(pend) == 1)
            # normalize: chain spread over scalar -> gpsimd -> vector -> gpsimd
            sums, bsums, recip = [], [], []
            for hh in range(2):
                s_ = small.tile([1, 512], f32, tag=f"sums{hh}", name=f"sums{hh}_{pair}_{J}")
                nc.scalar.copy(out=s_, in_=pa[hh][64:65, :])
                sums.append(s_)
            for hh in range(2):
                b_ = small.tile([64, 512], f32, tag=f"bsums{hh}", name=f"bsums{hh}_{pair}_{J}")
                nc.gpsimd.partition_broadcast(b_, sums[hh])
                bsums.append(b_)
            for hh in range(2):
                r_ = small.tile([64, 512], f32, tag=f"recip{hh}", name=f"recip{hh}_{pair}_{J}")
                nc.vector.reciprocal_approx_fast(out=r_, in_=bsums[hh])
                recip.append(r_)
            nc.vector.tensor_mul(attnT[pair][0:64, jsl], pa[0][0:64, :], recip[0])
            tmp = small.tile([64, 512], f16, tag="tmp")
            nc.vector.tensor_mul(tmp, pa[1][0:64, :], recip[1])
            nc.sync.dma_start(out=attnT[pair][64:128, jsl], in_=tmp)

        # ---------- phase A: left-half (tq/ts < 1024) producers ----------
        for J in (0, 1):
            qk_chain(0, qT[0], wq_sb, J, f"q0_{J}")
        for t in range(8):
            v_chain(t)
        for J in (0, 1):
            qk_chain(0, kT[0], wk_sb, J, f"k0_{J}")

        # ---------- phase B: attention(pair0); fillers = right-half qkv + qk(pair1) ----------
        for J in (2, 3):
            filler.extend(qk_chain_units(0, qT[0], wq_sb, J, f"q0_{J}"))
        for t in range(8, NTS):
            filler.append(lambda t=t: v_chain(t))
        for J in (2, 3):
            filler.extend(qk_chain_units(0, kT[0], wk_sb, J, f"k0_{J}"))
        for J in range(NJ):
            filler.extend(qk_chain_units(1, qT[1], wq_sb, J, f"q1_{J}"))
        for J in range(NJ):
            filler.extend(qk_chain_units(1, kT[1], wk_sb, J, f"k1_{J}"))
        # front-load: the J=0/1 blocks are small, so pull extra fillers there
        for J in range(NJ):
            att_pair_block(0, J, extra=3 if J < 2 else 2)
        pull(len(filler))

        # ---------- phase C: attention(pair1), gaps filled with proj ----------
        for J in range(NJ):
            att_pair_block(1, J, extra=2)
            filler.extend(
                (lambda m=m, n=n: (lambda: proj_tile(m, n)))()
                for m in range(4 * J, 4 * J + 4) for n in range(2))
        pull(len(filler))

    nc.compile()
    _NC_CACHE["nc"] = nc
    return nc


def make_in_maps(x, wq, wk, wv, wproj):
    def packw(w):  # (C, N) -> [128, C//128, N] -> flat [128, -1]
        n = w.shape[1]
        return np.ascontiguousarray(
            w.reshape(-1, 128, n).transpose(1, 0, 2).reshape(128, -1)
        ).astype(np.float16)

    xTs = [packw(np.ascontiguousarray(x[b].T).reshape(C, T)).reshape(128, NKC, T)
           for b in range(B)]
    in_maps = []
    for core in range(NCORES):
        b, g = divmod(core, 4)
        hs = slice(4 * g, 4 * g + 4)
        in_maps.append({
            "xt_p": xTs[b],
            "wq_p": packw(wq[hs].transpose(1, 0, 2).reshape(C, HPC * HS)),
            "wk_p": packw(wk[hs].transpose(1, 0, 2).reshape(C, HPC * HS)),
            "wv_p": packw(wv[hs].transpose(1, 0, 2).reshape(C, HPC * HS)),
            "wp_p": packw(wproj[4 * g * HS:(4 * g + 4) * HS, :]),
        })
    return in_maps


def _assemble(results, bproj):
    y = np.zeros((B, T, C), dtype=np.float32)
    for core in range(NCORES):
        yp = results[core]["y"].astype(np.float32)        # [128, 8, 2048]
        yp = yp.reshape(128, 8, 2, 1024).transpose(1, 2, 0, 3).reshape(T, C)
        y[core // 4] += yp
    y += bproj.astype(np.float32)[None, None, :]
    return y


def _is_causal(attention_mask):
    tril = np.tril(np.ones((T, T), dtype=bool))
    return all(np.array_equal(attention_mask[b], tril) for b in range(B))


def _numpy_fallback(x, attention_mask, wq, wk, wv, wproj, bproj):
    x64 = x.astype(np.float32)
    q = np.einsum('btc,hcd->bhtd', x64, wq)
    k = np.einsum('btc,hcd->bhtd', x64, wk)
    v = np.einsum('btc,hcd->bhtd', x64, wv)
    wei = np.einsum('bhtd,bhsd->bhts', q, k) / np.sqrt(np.float32(HS))
    wei = np.where(attention_mask[:, None, :, :], wei, -np.inf)
    wei = wei - wei.max(axis=-1, keepdims=True)
    wei = np.exp(wei)
    wei = wei / wei.sum(axis=-1, keepdims=True)
    out = np.einsum('bhts,bhsd->bhtd', wei, v)
    out = out.transpose(0, 2, 1, 3).reshape(B, T, H * HS)
    return (out @ wproj + bproj).astype(np.float32)


def _install_ntff_hook():
    """Recreate the antenv.axon_hooks shim so trace=True works under axon."""
    import sys, types
    try:
        from antenv.axon_hooks import get_axon_ntff_profile_hook  # noqa
        return
    except ImportError:
        pass
    import antenv
    mod = types.ModuleType("antenv.axon_hooks")
    holder = [None]
    mod.set_axon_ntff_profile_hook = lambda h: holder.__setitem__(0, h)
    mod.get_axon_ntff_profile_hook = lambda: holder[0]
    sys.modules["antenv.axon_hooks"] = mod
    antenv.axon_hooks = mod
    if "/root/.axon_site" not in sys.path:
        sys.path.insert(0, "/root/.axon_site")
    from trn_agent_boot.trn_boot import _ntff_profile_via_ctypes
    mod.set_axon_ntff_profile_hook(_ntff_profile_via_ctypes("/opt/axon/libaxon_pjrt.so"))


def kernel(x, attention_mask, wq, wk, wv, wproj, bproj, _trace=False):
    x = np.asarray(x); attention_mask = np.asarray(attention_mask)
    wq = np.asarray(wq); wk = np.asarray(wk); wv = np.asarray(wv)
    wproj = np.asarray(wproj); bproj = np.asarray(bproj)

    if not _is_causal(attention_mask):
        return _numpy_fallback(x, attention_mask, wq, wk, wv, wproj, bproj)

    from concourse import bass_utils
    if _trace:
        _install_ntff_hook()
        bass_utils.upload_artifacts = lambda d: d
    nc = _build_nc()
    in_maps = make_in_maps(x, wq, wk, wv, wproj)
    res = bass_utils.run_bass_kernel_spmd(
        nc, in_maps, core_ids=list(range(NCORES)), trace=_trace)
    out = _assemble(res.results, bproj)
    if _trace:
        return out, res
    return out


# revision 7
# speedup vs baseline: 1.0804x; 1.0804x over previous
"""Multi-head causal attention on 8 Trainium2 NeuronCores.

Problem: B=2, T=2048, C=1024, H=16, HS=64 (fp32), causal mask.

Sharding: 8 cores = 2 batches x 4 head-groups (4 heads each). Each core
computes q/k/v projections + attention + its partial output projection for
its 4 heads of its batch; the host sums the 4 per-batch partials (the
all-reduce of the tensor-parallel output projection) and adds the bias.

v4 design:
  * all matmul operands fp16 (1 cyc/row on PE, half the DMA/SBUF bytes);
    accumulation stays fp32 in PSUM, error ~5e-4 << 2e-2 gate.
  * scores row-tiled: head A contracts over PE rows 0-63, head B over rows
    64-127 concurrently (tile_position via base_partition).
  * DMA queues move ~114GB/s each and have ~6us start latency, so x is
    repacked host-side into [128, J, chunk, 512] J-column slices (8KB
    partition lines, one transfer per J) and phase A is J-sliced: the
    first attention block starts as soon as wq/wk/wv + x(J=0) land.
  * attention pipeline is exp-paced (scalar (N+352)/1.2); each block's
    normalize (sums->bcast->recip->mul) is emitted DEFERRED, inside the
    next block after its u=1, so it never queues ahead of the next exps;
    AV lags scores by 2 steps (pend deque) to cover the pa-bank reuse.
  * y out packed [128, 8, 2048] fp16 (host unpacks/sums), every transfer
    split in half across the sync and gpsimd queues.
"""

import numpy as np

B, T, C, H, HS = 2, 2048, 1024, 16, 64
NCORES = 8
HPC = 4            # heads per core
NKC = C // 128     # contraction chunks (8)
NJ = T // 512      # tq chunks (4)
NTS = T // 128     # ts chunks (16)

_NC_CACHE = {}


def _build_nc():
    if "nc" in _NC_CACHE:
        return _NC_CACHE["nc"]
    from contextlib import ExitStack
    import concourse.bass as bass
    from concourse import bacc, tile, mybir

    f32 = mybir.dt.float32
    f16 = mybir.dt.float16
    EXP = mybir.ActivationFunctionType.Exp

    nc = bacc.Bacc("TRN2", target_bir_lowering=False, debug=False,
                   enable_asserts=False, num_devices=NCORES)

    # prepacked inputs (per-partition-contiguous lines: x 8KB, weights 4KB)
    xt_d = nc.dram_tensor("xt_p", (128, NJ, NKC, 512), f16, kind="ExternalInput").ap()
    wq_d = nc.dram_tensor("wq_p", (128, NKC * 256), f16, kind="ExternalInput").ap()
    wk_d = nc.dram_tensor("wk_p", (128, NKC * 256), f16, kind="ExternalInput").ap()
    wv_d = nc.dram_tensor("wv_p", (128, NKC * 256), f16, kind="ExternalInput").ap()
    wp_d = nc.dram_tensor("wp_p", (128, 2 * C), f16, kind="ExternalInput").ap()
    # packed output: [p, mp, 1024*h + c] = y[128*(2*mp+h) + p, c]
    y_d = nc.dram_tensor("y", (128, 8, 2048), f16, kind="ExternalOutput").ap()

    scale = float(1.0 / np.sqrt(HS))

    with tile.TileContext(nc) as tc, ExitStack() as ctx:
        persist = ctx.enter_context(tc.tile_pool(name="persist", bufs=1))
        work = ctx.enter_context(tc.tile_pool(name="work", bufs=2))
        small = ctx.enter_context(tc.tile_pool(name="small", bufs=2))
        outp = ctx.enter_context(tc.tile_pool(name="outp", bufs=2))
        psS = ctx.enter_context(tc.tile_pool(name="psS", bufs=1, space="PSUM"))
        psatt = ctx.enter_context(tc.tile_pool(name="psatt", bufs=1, space="PSUM"))
        psaux = ctx.enter_context(tc.tile_pool(name="psaux", bufs=2, space="PSUM"))

        # ---- persistent SBUF tensors (fp16 matmul operands) ----
        xtJ = persist.tile([128, NJ, NKC, 512], f16, tag="xtJ")
        wq_sb = persist.tile([128, NKC, 256], f16, tag="wq")
        wk_sb = persist.tile([128, NKC, 256], f16, tag="wk")
        wv_sb = persist.tile([128, NKC, 256], f16, tag="wv")
        wp_sb = persist.tile([128, 2, C], f16, tag="wp")
        qT = [persist.tile([128, T], f16, tag=f"qT{p}", name=f"qT{p}") for p in range(2)]
        kT = [persist.tile([128, T], f16, tag=f"kT{p}", name=f"kT{p}") for p in range(2)]
        # v: per ts-chunk t and head h, [:, t, h, 0:64] = v dims, [..., 64] = 1.0
        vt = persist.tile([128, NTS, HPC, 65], f16, tag="vt")
        attnT = [persist.tile([128, T], f16, tag=f"attnT{p}", name=f"attnT{p}") for p in range(2)]

        # ---- loads: consumption order; sync/gpsimd/scalar queues ----
        wqf = wq_sb.rearrange("p k n -> p (k n)")
        wkf = wk_sb.rearrange("p k n -> p (k n)")
        wvf = wv_sb.rearrange("p k n -> p (k n)")
        wpf = wp_sb.rearrange("p k n -> p (k n)")
        nc.sync.dma_start(out=xtJ[:, 0, :, :], in_=xt_d[:, 0, :, :])
        nc.scalar.dma_start(out=wqf, in_=wq_d)
        nc.gpsimd.dma_start(out=wvf, in_=wv_d)
        nc.scalar.dma_start(out=wkf, in_=wk_d)
        nc.gpsimd.dma_start(out=xtJ[:, 1, :, :], in_=xt_d[:, 1, :, :])
        nc.scalar.dma_start(out=xtJ[:, 2, :, :], in_=xt_d[:, 2, :, :])
        nc.sync.dma_start(out=xtJ[:, 3, :, :], in_=xt_d[:, 3, :, :])
        nc.sync.dma_start(out=wpf, in_=wp_d)

        # exp table preload (after the load triggers so those fire first)
        dummy = small.tile([1, 2], f32, tag="dummy")
        nc.vector.memset(dummy, 0.0)
        nc.scalar.activation(out=dummy[0:1, 1:2], in_=dummy[0:1, 0:1], func=EXP)

        nc.vector.memset(vt[:, :, :, 64:65], 1.0)

        # ---------- emission helpers ----------
        filler = []     # queue of closures emitting independent PE work

        def pull(n):
            for _ in range(n):
                if filler:
                    filler.pop(0)()

        def qk_chain_units(pair, dst, w_sb, J, name):
            # split one 8-matmul accumulation chain into 4 filler units
            ps = psaux.tile([128, 512], f32, tag="aux", name=name)

            def unit(c0):
                def f():
                    for c in (c0, c0 + 1):
                        nc.tensor.matmul(
                            ps,
                            lhsT=w_sb[:, c, 128 * pair:128 * pair + 128],
                            rhs=xtJ[:, J, c, :],
                            start=(c == 0), stop=(c == NKC - 1))
                    if c0 == NKC - 2:
                        nc.vector.tensor_copy(
                            out=dst[:, 512 * J:512 * J + 512], in_=ps)
                return f
            return [unit(c0) for c0 in range(0, NKC, 2)]

        def qk_chain(pair, dst, w_sb, J, name):
            for u in qk_chain_units(pair, dst, w_sb, J, name):
                u()

        def v_chain(t):
            ps = psaux.tile([128, 512], f32, tag="aux", name=f"v_{t}")
            Jb, r = divmod(t, 4)
            for c in range(NKC):
                nc.tensor.matmul(
                    ps[:, 0:256],
                    lhsT=xtJ[:, Jb, c, 128 * r:128 * r + 128],
                    rhs=wv_sb[:, c, :],
                    start=(c == 0), stop=(c == NKC - 1))
            nc.vector.tensor_copy(out=vt[:, t, :, 0:64], in_=ps[:, 0:256])

        yo2 = {}

        def proj_tile(m, n):
            # proj for row-block m, col-half n; packed into [128,2048] pair
            # tiles (mp = m//2); each DMAs out as two halves on both queues.
            mp, h = divmod(m, 2)
            py_ = psaux.tile([128, 512], f32, tag="aux", name=f"y_{m}_{n}")
            for pair in range(2):
                nc.tensor.matmul(
                    py_,
                    lhsT=attnT[pair][:, 128 * m:128 * m + 128],
                    rhs=wp_sb[:, pair, 512 * n:512 * n + 512],
                    start=(pair == 0), stop=(pair == 1))
            if mp not in yo2:
                yo2[mp] = outp.tile([128, 2048], f16, tag=f"yo{mp % 2}",
                                    name=f"yo2_{mp}")
            nc.vector.tensor_copy(
                out=yo2[mp][:, 1024 * h + 512 * n:1024 * h + 512 * n + 512],
                in_=py_)
            if h == 1 and n == 1:
                t_ = yo2.pop(mp)
                nc.sync.dma_start(out=y_d[:, mp, 0:1024], in_=t_[:, 0:1024])
                nc.gpsimd.dma_start(out=y_d[:, mp, 1024:2048], in_=t_[:, 1024:2048])

        def att_pair_block(pair, J, extra=2, pending=None):
            """Attention for both heads of `pair` over tq block J.

            Returns a `finalize` closure (normalize + attnT writeback) that
            the caller passes into the NEXT block as `pending`, so its
            cross-engine chain is emitted after that block's u=1 and never
            blocks the next exps at a block boundary.
            """
            nch = 4 * J + 4
            jsl = slice(512 * J, 512 * J + 512)
            pa = [psatt.tile([65, 512], f32, tag=f"pa{hh}", name=f"pa{hh}_{pair}_{J}")
                  for hh in range(2)]
            pend = []          # AV two steps behind scores

            def flush_av(last=False):
                etA, etB, t0, t1 = pend.pop(0)
                for half, t in ((0, t0), (1, t1)):
                    for hh, et in ((0, etA), (1, etB)):
                        nc.tensor.matmul(
                            pa[hh], lhsT=vt[:, t, 2 * pair + hh, :],
                            rhs=et[:, 512 * half:512 * half + 512],
                            start=(t == 0),
                            stop=(last and t == nch - 1))

            for u in range(nch // 2):
                t0, t1 = 2 * u, 2 * u + 1
                ss = [psS.tile([128, 1024], f32, tag=f"s{hh}", name=f"ss{hh}_{pair}_{J}_{u}")
                      for hh in range(2)]
                for half, t in ((0, t0), (1, t1)):
                    for hh in range(2):
                        hsl = slice(64 * hh, 64 * hh + 64)
                        nc.tensor.matmul(
                            ss[hh][:, 512 * half:512 * half + 512],
                            lhsT=kT[pair][hsl, 128 * t:128 * t + 128],
                            rhs=qT[pair][hsl, jsl],
                            start=True, stop=True)
                et = [work.tile([128, 1024], f16, tag=f"et{hh}", bufs=3,
                                name=f"et{hh}_{pair}_{J}_{u}")
                      for hh in range(2)]
                for hh in range(2):
                    nc.scalar.activation(out=et[hh], in_=ss[hh], func=EXP, scale=scale)
                for half, t in ((0, t0), (1, t1)):
                    if t >= 4 * J:
                        d = t - 4 * J
                        for hh in range(2):
                            sl = et[hh][:, 512 * half:512 * half + 512]
                            # keep el iff f >= p + 128*d
                            nc.gpsimd.affine_select(
                                out=sl, in_=sl,
                                compare_op=mybir.AluOpType.is_ge,
                                fill=0.0, base=-128 * d,
                                pattern=[[1, 512]], channel_multiplier=-1)
                pend.append((et[0], et[1], t0, t1))
                if len(pend) > 2:
                    flush_av()
                pull(extra)
                if u == min(1, nch // 2 - 1) and pending is not None:
                    pending()
            while pend:
                flush_av(last=len(pend) == 1)

            def finalize():
                sums, bsums, recip = [], [], []
                for hh in range(2):
                    s_ = small.tile([1, 512], f32, tag=f"sums{hh}",
                                    name=f"sums{hh}_{pair}_{J}")
                    nc.scalar.copy(out=s_, in_=pa[hh][64:65, :])
                    sums.append(s_)
                for hh in range(2):
                    b_ = small.tile([64, 512], f32, tag=f"bsums{hh}",
                                    name=f"bsums{hh}_{pair}_{J}")
                    nc.gpsimd.partition_broadcast(b_, sums[hh])
                    bsums.append(b_)
                for hh in range(2):
                    r_ = small.tile([64, 512], f32, tag=f"recip{hh}",
                                    name=f"recip{hh}_{pair}_{J}")
                    nc.vector.reciprocal_approx_fast(out=r_, in_=bsums[hh])
                    recip.append(r_)
                nc.vector.tensor_mul(attnT[pair][0:64, jsl], pa[0][0:64, :], recip[0])
                tmp = small.tile([64, 512], f16, tag="tmp")
                nc.vector.tensor_mul(tmp, pa[1][0:64, :], recip[1])
                nc.sync.dma_start(out=attnT[pair][64:128, jsl], in_=tmp)
            return finalize

        # ---------- J-sliced schedule: attention starts once x(J=0) lands ----
        qk_chain(0, qT[0], wq_sb, 0, "q0_0")
        qk_chain(0, kT[0], wk_sb, 0, "k0_0")
        for t in range(4):
            v_chain(t)

        filler.extend(qk_chain_units(0, qT[0], wq_sb, 1, "q0_1"))
        filler.extend(qk_chain_units(0, kT[0], wk_sb, 1, "k0_1"))
        for t in range(4, 8):
            filler.append(lambda t=t: v_chain(t))
        fin = att_pair_block(0, 0, extra=3)
        pull(len(filler))

        filler.extend(qk_chain_units(0, qT[0], wq_sb, 2, "q0_2"))
        filler.extend(qk_chain_units(0, kT[0], wk_sb, 2, "k0_2"))
        for t in range(8, 12):
            filler.append(lambda t=t: v_chain(t))
        filler.extend(qk_chain_units(1, qT[1], wq_sb, 0, "q1_0"))
        filler.extend(qk_chain_units(1, kT[1], wk_sb, 0, "k1_0"))
        fin = att_pair_block(0, 1, extra=3, pending=fin)
        pull(8)

        filler.extend(qk_chain_units(0, qT[0], wq_sb, 3, "q0_3"))
        filler.extend(qk_chain_units(0, kT[0], wk_sb, 3, "k0_3"))
        for t in range(12, 16):
            filler.append(lambda t=t: v_chain(t))
        filler.extend(qk_chain_units(1, qT[1], wq_sb, 1, "q1_1"))
        filler.extend(qk_chain_units(1, kT[1], wk_sb, 1, "k1_1"))
        fin = att_pair_block(0, 2, extra=2, pending=fin)
        pull(8)

        for J in (2, 3):
            filler.extend(qk_chain_units(1, qT[1], wq_sb, J, f"q1_{J}"))
            filler.extend(qk_chain_units(1, kT[1], wk_sb, J, f"k1_{J}"))
        fin = att_pair_block(0, 3, extra=2, pending=fin)
        pull(len(filler))

        # ---------- pair 1 attention; gaps filled with proj ----------
        for J in range(NJ):
            fin = att_pair_block(1, J, extra=2, pending=fin)
            if J > 0:   # proj for the previous J (its finalize just emitted)
                filler.extend(
                    (lambda m=m, n=n: (lambda: proj_tile(m, n)))()
                    for m in range(4 * (J - 1), 4 * (J - 1) + 4) for n in range(2))
        fin()
        for m in range(12, 16):
            for n in range(2):
                filler.append((lambda m=m, n=n: (lambda: proj_tile(m, n)))())
        pull(len(filler))

    nc.compile()
    _NC_CACHE["nc"] = nc
    return nc


def make_in_maps(x, wq, wk, wv, wproj):
    def packw(w):  # (C, N) -> [128, C//128, N] -> flat [128, -1]
        n = w.shape[1]
        return np.ascontiguousarray(
            w.reshape(-1, 128, n).transpose(1, 0, 2).reshape(128, -1)
        ).astype(np.float16)

    # x[b] (T, C) -> xT (C, T) -> [c, p, Jb, col] -> [p, Jb, c, col]
    xTs = [np.ascontiguousarray(
               x[b].T.reshape(NKC, 128, NJ, 512).transpose(1, 2, 0, 3)
           ).astype(np.float16)
           for b in range(B)]
    in_maps = []
    for core in range(NCORES):
        b, g = divmod(core, 4)
        hs = slice(4 * g, 4 * g + 4)
        in_maps.append({
            "xt_p": xTs[b],
            "wq_p": packw(wq[hs].transpose(1, 0, 2).reshape(C, HPC * HS)),
            "wk_p": packw(wk[hs].transpose(1, 0, 2).reshape(C, HPC * HS)),
            "wv_p": packw(wv[hs].transpose(1, 0, 2).reshape(C, HPC * HS)),
            "wp_p": packw(wproj[4 * g * HS:(4 * g + 4) * HS, :]),
        })
    return in_maps


def _assemble(results, bproj):
    y = np.zeros((B, T, C), dtype=np.float32)
    for core in range(NCORES):
        yp = results[core]["y"].astype(np.float32)        # [128, 8, 2048]
        yp = yp.reshape(128, 8, 2, 1024).transpose(1, 2, 0, 3).reshape(T, C)
        y[core // 4] += yp
    y += bproj.astype(np.float32)[None, None, :]
    return y


def _is_causal(attention_mask):
    tril = np.tril(np.ones((T, T), dtype=bool))
    return all(np.array_equal(attention_mask[b], tril) for b in range(B))


def _numpy_fallback(x, attention_mask, wq, wk, wv, wproj, bproj):
    x64 = x.astype(np.float32)
    q = np.einsum('btc,hcd->bhtd', x64, wq)
    k = np.einsum('btc,hcd->bhtd', x64, wk)
    v = np.einsum('btc,hcd->bhtd', x64, wv)
    wei = np.einsum('bhtd,bhsd->bhts', q, k) / np.sqrt(np.float32(HS))
    wei = np.where(attention_mask[:, None, :, :], wei, -np.inf)
    wei = wei - wei.max(axis=-1, keepdims=True)
    wei = np.exp(wei)
    wei = wei / wei.sum(axis=-1, keepdims=True)
    out = np.einsum('bhts,bhsd->bhtd', wei, v)
    out = out.transpose(0, 2, 1, 3).reshape(B, T, H * HS)
    return (out @ wproj + bproj).astype(np.float32)


def _install_ntff_hook():
    """Recreate the antenv.axon_hooks shim so trace=True works under axon."""
    import sys, types
    try:
        from antenv.axon_hooks import get_axon_ntff_profile_hook  # noqa
        return
    except ImportError:
        pass
    import antenv
    mod = types.ModuleType("antenv.axon_hooks")
    holder = [None]
    mod.set_axon_ntff_profile_hook = lambda h: holder.__setitem__(0, h)
    mod.get_axon_ntff_profile_hook = lambda: holder[0]
    sys.modules["antenv.axon_hooks"] = mod
    antenv.axon_hooks = mod
    if "/root/.axon_site" not in sys.path:
        sys.path.insert(0, "/root/.axon_site")
    from trn_agent_boot.trn_boot import _ntff_profile_via_ctypes
    mod.set_axon_ntff_profile_hook(_ntff_profile_via_ctypes("/opt/axon/libaxon_pjrt.so"))


def kernel(x, attention_mask, wq, wk, wv, wproj, bproj, _trace=False):
    x = np.asarray(x); attention_mask = np.asarray(attention_mask)
    wq = np.asarray(wq); wk = np.asarray(wk); wv = np.asarray(wv)
    wproj = np.asarray(wproj); bproj = np.asarray(bproj)

    if not _is_causal(attention_mask):
        return _numpy_fallback(x, attention_mask, wq, wk, wv, wproj, bproj)

    from concourse import bass_utils
    if _trace:
        _install_ntff_hook()
        bass_utils.upload_artifacts = lambda d: d
    nc = _build_nc()
    in_maps = make_in_maps(x, wq, wk, wv, wproj)
    res = bass_utils.run_bass_kernel_spmd(
        nc, in_maps, core_ids=list(range(NCORES)), trace=_trace)
    out = _assemble(res.results, bproj)
    if _trace:
        return out, res
    return out


# revision 13
# speedup vs baseline: 1.1329x; 1.0486x over previous
"""Multi-head causal attention on 8 Trainium2 NeuronCores.

Problem: B=2, T=2048, C=1024, H=16, HS=64 (fp32), causal mask.

Sharding: 8 cores = 2 batches x 4 head-groups (4 heads each). Each core
computes q/k/v projections + attention + its partial output projection for
its 4 heads of its batch; the host sums the 4 per-batch partials (the
all-reduce of the tensor-parallel output projection) and adds the bias.

v4 design:
  * all matmul operands fp16 (1 cyc/row on PE, half the DMA/SBUF bytes);
    accumulation stays fp32 in PSUM, error ~5e-4 << 2e-2 gate.
  * scores row-tiled: head A contracts over PE rows 0-63, head B over rows
    64-127 concurrently (tile_position via base_partition).
  * DMA queues move ~114GB/s each and have ~6us start latency, so x is
    repacked host-side into [128, J, chunk, 512] J-column slices (8KB
    partition lines, one transfer per J) and phase A is J-sliced: the
    first attention block starts as soon as wq/wk/wv + x(J=0) land.
  * attention pipeline is exp-paced (scalar (N+352)/1.2); each block's
    normalize (sums->bcast->recip->mul) is emitted DEFERRED, inside the
    next block after its u=1, so it never queues ahead of the next exps;
    AV lags scores by 2 steps (pend deque) to cover the pa-bank reuse.
  * y out packed [128, 8, 2048] fp16 (host unpacks/sums), every transfer
    split in half across the sync and gpsimd queues.
"""

import numpy as np

B, T, C, H, HS = 2, 2048, 1024, 16, 64
NCORES = 8
HPC = 4            # heads per core
NKC = C // 128     # contraction chunks (8)
NJ = T // 512      # tq chunks (4)
NTS = T // 128     # ts chunks (16)

_NC_CACHE = {}


def _build_nc():
    if "nc" in _NC_CACHE:
        return _NC_CACHE["nc"]
    from contextlib import ExitStack
    import concourse.bass as bass
    from concourse import bacc, tile, mybir

    f32 = mybir.dt.float32
    f16 = mybir.dt.float16
    EXP = mybir.ActivationFunctionType.Exp

    nc = bacc.Bacc("TRN2", target_bir_lowering=False, debug=False,
                   enable_asserts=False, num_devices=NCORES)

    # prepacked inputs (per-partition-contiguous lines: x 8KB, weights 4KB)
    xt_d = nc.dram_tensor("xt_p", (128, NJ, NKC, 512), f16, kind="ExternalInput").ap()
    wq_d = nc.dram_tensor("wq_p", (128, NKC * 256), f16, kind="ExternalInput").ap()
    wk_d = nc.dram_tensor("wk_p", (128, NKC * 256), f16, kind="ExternalInput").ap()
    wv_d = nc.dram_tensor("wv_p", (128, NKC * 256), f16, kind="ExternalInput").ap()
    wp_d = nc.dram_tensor("wp_p", (128, 2 * C), f16, kind="ExternalInput").ap()
    # packed output: [p, mp, 1024*h + c] = y[128*(2*mp+h) + p, c]
    y_d = nc.dram_tensor("y", (128, 8, 2048), f16, kind="ExternalOutput").ap()

    scale = float(1.0 / np.sqrt(HS))

    with tile.TileContext(nc) as tc, ExitStack() as ctx:
        persist = ctx.enter_context(tc.tile_pool(name="persist", bufs=1))
        work = ctx.enter_context(tc.tile_pool(name="work", bufs=2))
        small = ctx.enter_context(tc.tile_pool(name="small", bufs=2))
        outp = ctx.enter_context(tc.tile_pool(name="outp", bufs=2))
        psS = ctx.enter_context(tc.tile_pool(name="psS", bufs=1, space="PSUM"))
        psatt = ctx.enter_context(tc.tile_pool(name="psatt", bufs=1, space="PSUM"))
        psaux = ctx.enter_context(tc.tile_pool(name="psaux", bufs=2, space="PSUM"))

        # ---- persistent SBUF tensors (fp16 matmul operands) ----
        xtJ = persist.tile([128, NJ, NKC, 512], f16, tag="xtJ")
        wq_sb = persist.tile([128, NKC, 256], f16, tag="wq")
        wk_sb = persist.tile([128, NKC, 256], f16, tag="wk")
        wv_sb = persist.tile([128, NKC, 256], f16, tag="wv")
        wp_sb = persist.tile([128, 2, C], f16, tag="wp")
        qT = [persist.tile([128, T], f16, tag=f"qT{p}", name=f"qT{p}") for p in range(2)]
        kT = [persist.tile([128, T], f16, tag=f"kT{p}", name=f"kT{p}") for p in range(2)]
        # v: per ts-chunk t and head h, [:, t, h, 0:64] = v dims, [..., 64] = 1.0
        vt = persist.tile([128, NTS, HPC, 65], f16, tag="vt")
        attnT = [persist.tile([128, T], f16, tag=f"attnT{p}", name=f"attnT{p}") for p in range(2)]

        # ---- loads: consumption order; sync/gpsimd/scalar queues ----
        wqf = wq_sb.rearrange("p k n -> p (k n)")
        wkf = wk_sb.rearrange("p k n -> p (k n)")
        wvf = wv_sb.rearrange("p k n -> p (k n)")
        wpf = wp_sb.rearrange("p k n -> p (k n)")
        # xJ0 split across sync+gpsimd so the first chain starts ~6us sooner
        nc.sync.dma_start(out=xtJ[:, 0, 0:4, :], in_=xt_d[:, 0, 0:4, :])
        nc.gpsimd.dma_start(out=xtJ[:, 0, 4:8, :], in_=xt_d[:, 0, 4:8, :])
        nc.scalar.dma_start(out=wqf, in_=wq_d)
        nc.gpsimd.dma_start(out=wvf, in_=wv_d)
        nc.scalar.dma_start(out=wkf, in_=wk_d)
        nc.sync.dma_start(out=xtJ[:, 1, :, :], in_=xt_d[:, 1, :, :])
        nc.gpsimd.dma_start(out=xtJ[:, 2, :, :], in_=xt_d[:, 2, :, :])
        nc.scalar.dma_start(out=xtJ[:, 3, :, :], in_=xt_d[:, 3, :, :])
        nc.sync.dma_start(out=wpf, in_=wp_d)

        # exp table preload (after the load triggers so those fire first)
        dummy = small.tile([1, 2], f32, tag="dummy")
        nc.vector.memset(dummy, 0.0)
        nc.scalar.activation(out=dummy[0:1, 1:2], in_=dummy[0:1, 0:1], func=EXP)

        nc.vector.memset(vt[:, :, :, 64:65], 1.0)

        # ---------- emission helpers ----------
        filler = []     # queue of closures emitting independent PE work
        pulled = [0]

        def pull(n):
            for _ in range(n):
                if filler:
                    filler.pop(0)()
                    pulled[0] += 1

        def qk_chain_units(pair, dst, w_sb, J, name):
            # split one 8-matmul accumulation chain into 4 filler units
            ps = psaux.tile([128, 512], f32, tag="aux", name=name)

            def unit(c0):
                def f():
                    for c in (c0, c0 + 1):
                        nc.tensor.matmul(
                            ps,
                            lhsT=w_sb[:, c, 128 * pair:128 * pair + 128],
                            rhs=xtJ[:, J, c, :],
                            start=(c == 0), stop=(c == NKC - 1))
                    if c0 == NKC - 2:
                        nc.vector.tensor_copy(
                            out=dst[:, 512 * J:512 * J + 512], in_=ps)
                return f
            return [unit(c0) for c0 in range(0, NKC, 2)]

        def qk_chain(pair, dst, w_sb, J, name):
            for u in qk_chain_units(pair, dst, w_sb, J, name):
                u()

        def v_chain(t):
            ps = psaux.tile([128, 512], f32, tag="aux", name=f"v_{t}")
            Jb, r = divmod(t, 4)
            for c in range(NKC):
                nc.tensor.matmul(
                    ps[:, 0:256],
                    lhsT=xtJ[:, Jb, c, 128 * r:128 * r + 128],
                    rhs=wv_sb[:, c, :],
                    start=(c == 0), stop=(c == NKC - 1))
            nc.vector.tensor_copy(out=vt[:, t, :, 0:64], in_=ps[:, 0:256])

        yo2 = {}

        def proj_tile(m, n):
            # proj for row-block m, col-half n; packed into [128,2048] pair
            # tiles (mp = m//2); each DMAs out as two halves on both queues.
            mp, h = divmod(m, 2)
            py_ = psaux.tile([128, 512], f32, tag="aux", name=f"y_{m}_{n}")
            for pair in range(2):
                nc.tensor.matmul(
                    py_,
                    lhsT=attnT[pair][:, 128 * m:128 * m + 128],
                    rhs=wp_sb[:, pair, 512 * n:512 * n + 512],
                    start=(pair == 0), stop=(pair == 1))
            if mp not in yo2:
                yo2[mp] = outp.tile([128, 2048], f16, tag=f"yo{mp % 2}",
                                    name=f"yo2_{mp}")
            nc.vector.tensor_copy(
                out=yo2[mp][:, 1024 * h + 512 * n:1024 * h + 512 * n + 512],
                in_=py_)
            if h == 1 and n == 1:
                t_ = yo2.pop(mp)
                nc.sync.dma_start(out=y_d[:, mp, 0:1024], in_=t_[:, 0:1024])
                nc.gpsimd.dma_start(out=y_d[:, mp, 1024:2048], in_=t_[:, 1024:2048])

        def att_pair_block(pair, J, extra=2, pending=None, after_pending=None):
            """Attention for both heads of `pair` over tq block J.

            Returns a `finalize` closure (normalize + attnT writeback) that
            the caller passes into the NEXT block as `pending`, so its
            cross-engine chain is emitted after that block's u=1 and never
            blocks the next exps at a block boundary.
            """
            nch = 4 * J + 4
            jsl = slice(512 * J, 512 * J + 512)
            pa = [psatt.tile([65, 512], f32, tag=f"pa{hh}", name=f"pa{hh}_{pair}_{J}")
                  for hh in range(2)]
            pend = []          # AV two steps behind scores

            def flush_av(last=False):
                etA, etB, t0, t1 = pend.pop(0)
                for half, t in ((0, t0), (1, t1)):
                    for hh, et in ((0, etA), (1, etB)):
                        nc.tensor.matmul(
                            pa[hh], lhsT=vt[:, t, 2 * pair + hh, :],
                            rhs=et[:, 512 * half:512 * half + 512],
                            start=(t == 0),
                            stop=(last and t == nch - 1))

            for u in range(nch // 2):
                t0, t1 = 2 * u, 2 * u + 1
                ss = [psS.tile([128, 1024], f32, tag=f"s{hh}", name=f"ss{hh}_{pair}_{J}_{u}")
                      for hh in range(2)]
                for half, t in ((0, t0), (1, t1)):
                    for hh in range(2):
                        hsl = slice(64 * hh, 64 * hh + 64)
                        nc.tensor.matmul(
                            ss[hh][:, 512 * half:512 * half + 512],
                            lhsT=kT[pair][hsl, 128 * t:128 * t + 128],
                            rhs=qT[pair][hsl, jsl],
                            start=True, stop=True)
                et = [work.tile([128, 1024], f16, tag=f"et{hh}", bufs=3,
                                name=f"et{hh}_{pair}_{J}_{u}")
                      for hh in range(2)]
                for hh in range(2):
                    nc.scalar.activation(out=et[hh], in_=ss[hh], func=EXP, scale=scale)
                for half, t in ((0, t0), (1, t1)):
                    if t >= 4 * J:
                        d = t - 4 * J
                        for hh in range(2):
                            sl = et[hh][:, 512 * half:512 * half + 512]
                            # keep el iff f >= p + 128*d
                            nc.gpsimd.affine_select(
                                out=sl, in_=sl,
                                compare_op=mybir.AluOpType.is_ge,
                                fill=0.0, base=-128 * d,
                                pattern=[[1, 512]], channel_multiplier=-1)
                pend.append((et[0], et[1], t0, t1))
                if len(pend) > 2:
                    flush_av()
                pull(extra)
                if u == min(1, nch // 2 - 1):
                    if pending is not None:
                        pending()
                    if after_pending:
                        filler.extend(after_pending)
            while pend:
                flush_av(last=len(pend) == 1)

            def finalize():
                sums, bsums, recip = [], [], []
                for hh in range(2):
                    s_ = small.tile([1, 512], f32, tag=f"sums{hh}",
                                    name=f"sums{hh}_{pair}_{J}")
                    nc.scalar.copy(out=s_, in_=pa[hh][64:65, :])
                    sums.append(s_)
                for hh in range(2):
                    b_ = small.tile([64, 512], f32, tag=f"bsums{hh}",
                                    name=f"bsums{hh}_{pair}_{J}")
                    nc.gpsimd.partition_broadcast(b_, sums[hh])
                    bsums.append(b_)
                for hh in range(2):
                    r_ = small.tile([64, 512], f32, tag=f"recip{hh}",
                                    name=f"recip{hh}_{pair}_{J}")
                    nc.vector.reciprocal_approx_fast(out=r_, in_=bsums[hh])
                    recip.append(r_)
                nc.vector.tensor_mul(attnT[pair][0:64, jsl], pa[0][0:64, :], recip[0])
                tmp = small.tile([64, 512], f16, tag="tmp")
                nc.vector.tensor_mul(tmp, pa[1][0:64, :], recip[1])
                nc.sync.dma_start(out=attnT[pair][64:128, jsl], in_=tmp)
            return finalize

        # ---------- J-sliced schedule: attention starts once x(J=0) lands ----
        qk_chain(0, qT[0], wq_sb, 0, "q0_0")
        qk_chain(0, kT[0], wk_sb, 0, "k0_0")
        for t in range(4):
            v_chain(t)

        # global ordered filler list: each block J's hard prerequisite is
        # q(J) fully emitted before its first scores; k(J)/v(J-range) are
        # consumed mid-block and ride the in-block pulls. `need[key]` is the
        # prefix of the list that must be emitted before that block starts.
        added = [0]
        need = {}

        def add_units(units):
            filler.extend(units)
            added[0] += len(units)

        for J in (1, 2, 3):
            add_units(qk_chain_units(0, qT[0], wq_sb, J, f"q0_{J}"))
            if J == 1:
                need[(0, 1)] = added[0]
            add_units(qk_chain_units(0, kT[0], wk_sb, J, f"k0_{J}"))
            add_units([(lambda t=t: v_chain(t)) for t in range(4 * J, 4 * J + 4)])
            if J == 2:
                need[(0, 2)] = added[0] - 8
            if J == 3:
                need[(0, 3)] = added[0] - 8
        for J in range(NJ):
            add_units(qk_chain_units(1, qT[1], wq_sb, J, f"q1_{J}"))
            if J > 0:   # k1(J) rides the in-block pulls; k1(0) is needed at u0
                need[(1, J)] = added[0]
            add_units(qk_chain_units(1, kT[1], wk_sb, J, f"k1_{J}"))
            if J == 0:
                need[(1, J)] = added[0]

        def ensure(key):
            pull(max(0, need[key] - pulled[0]))

        fin = att_pair_block(0, 0, extra=3)
        proj_of = {
            J: [(lambda m=m, n=n: (lambda: proj_tile(m, n)))()
                for m in range(4 * J, 4 * J + 4) for n in range(2)]
            for J in range(NJ)}
        for J in (1, 2, 3):
            ensure((0, J))
            fin = att_pair_block(0, J, extra=3 if J == 1 else 2, pending=fin)

        # ---------- pair 1 attention; gaps filled with proj ----------
        for J in range(NJ):
            ensure((1, J))
            fin = att_pair_block(1, J, extra=2 if J < 3 else 3, pending=fin,
                                 after_pending=proj_of[J - 1] if J > 0 else None)
        fin()
        for u_ in proj_of[3]:
            u_()
        pull(len(filler))

    nc.compile()
    _NC_CACHE["nc"] = nc
    return nc


def make_in_maps(x, wq, wk, wv, wproj):
    def packw(w):  # (C, N) -> [128, C//128, N] -> flat [128, -1]
        n = w.shape[1]
        return np.ascontiguousarray(
            w.reshape(-1, 128, n).transpose(1, 0, 2).reshape(128, -1)
        ).astype(np.float16)

    # x[b] (T, C) -> xT (C, T) -> [c, p, Jb, col] -> [p, Jb, c, col]
    xTs = [np.ascontiguousarray(
               x[b].T.reshape(NKC, 128, NJ, 512).transpose(1, 2, 0, 3)
           ).astype(np.float16)
           for b in range(B)]
    in_maps = []
    for core in range(NCORES):
        b, g = divmod(core, 4)
        hs = slice(4 * g, 4 * g + 4)
        in_maps.append({
            "xt_p": xTs[b],
            "wq_p": packw(wq[hs].transpose(1, 0, 2).reshape(C, HPC * HS)),
            "wk_p": packw(wk[hs].transpose(1, 0, 2).reshape(C, HPC * HS)),
            "wv_p": packw(wv[hs].transpose(1, 0, 2).reshape(C, HPC * HS)),
            "wp_p": packw(wproj[4 * g * HS:(4 * g + 4) * HS, :]),
        })
    return in_maps


def _assemble(results, bproj):
    y = np.zeros((B, T, C), dtype=np.float32)
    for core in range(NCORES):
        yp = results[core]["y"].astype(np.float32)        # [128, 8, 2048]
        yp = yp.reshape(128, 8, 2, 1024).transpose(1, 2, 0, 3).reshape(T, C)
        y[core // 4] += yp
    y += bproj.astype(np.float32)[None, None, :]
    return y


def _is_causal(attention_mask):
    tril = np.tril(np.ones((T, T), dtype=bool))
    return all(np.array_equal(attention_mask[b], tril) for b in range(B))


def _numpy_fallback(x, attention_mask, wq, wk, wv, wproj, bproj):
    x64 = x.astype(np.float32)
    q = np.einsum('btc,hcd->bhtd', x64, wq)
    k = np.einsum('btc,hcd->bhtd', x64, wk)
    v = np.einsum('btc,hcd->bhtd', x64, wv)
    wei = np.einsum('bhtd,bhsd->bhts', q, k) / np.sqrt(np.float32(HS))
    wei = np.where(attention_mask[:, None, :, :], wei, -np.inf)
    wei = wei - wei.max(axis=-1, keepdims=True)
    wei = np.exp(wei)
    wei = wei / wei.sum(axis=-1, keepdims=True)
    out = np.einsum('bhts,bhsd->bhtd', wei, v)
    out = out.transpose(0, 2, 1, 3).reshape(B, T, H * HS)
    return (out @ wproj + bproj).astype(np.float32)


def _install_ntff_hook():
    """Recreate the antenv.axon_hooks shim so trace=True works under axon."""
    import sys, types
    try:
        from antenv.axon_hooks import get_axon_ntff_profile_hook  # noqa
        return
    except ImportError:
        pass
    import antenv
    mod = types.ModuleType("antenv.axon_hooks")
    holder = [None]
    mod.set_axon_ntff_profile_hook = lambda h: holder.__setitem__(0, h)
    mod.get_axon_ntff_profile_hook = lambda: holder[0]
    sys.modules["antenv.axon_hooks"] = mod
    antenv.axon_hooks = mod
    if "/root/.axon_site" not in sys.path:
        sys.path.insert(0, "/root/.axon_site")
    from trn_agent_boot.trn_boot import _ntff_profile_via_ctypes
    mod.set_axon_ntff_profile_hook(_ntff_profile_via_ctypes("/opt/axon/libaxon_pjrt.so"))


def kernel(x, attention_mask, wq, wk, wv, wproj, bproj, _trace=False):
    x = np.asarray(x); attention_mask = np.asarray(attention_mask)
    wq = np.asarray(wq); wk = np.asarray(wk); wv = np.asarray(wv)
    wproj = np.asarray(wproj); bproj = np.asarray(bproj)

    if not _is_causal(attention_mask):
        return _numpy_fallback(x, attention_mask, wq, wk, wv, wproj, bproj)

    from concourse import bass_utils
    if _trace:
        _install_ntff_hook()
        bass_utils.upload_artifacts = lambda d: d
    nc = _build_nc()
    in_maps = make_in_maps(x, wq, wk, wv, wproj)
    res = bass_utils.run_bass_kernel_spmd(
        nc, in_maps, core_ids=list(range(NCORES)), trace=_trace)
    out = _assemble(res.results, bproj)
    if _trace:
        return out, res
    return out


# revision 16
# speedup vs baseline: 1.1990x; 1.0584x over previous
"""Multi-head causal attention on 8 Trainium2 NeuronCores.

Problem: B=2, T=2048, C=1024, H=16, HS=64 (fp32), causal mask.

Sharding: 8 cores = 2 batches x 4 head-groups (4 heads each). Each core
computes q/k/v projections + attention + its partial output projection for
its 4 heads of its batch; the host sums the 4 per-batch partials (the
all-reduce of the tensor-parallel output projection) and adds the bias.

v4 design:
  * all matmul operands fp16 (1 cyc/row on PE, half the DMA/SBUF bytes);
    accumulation stays fp32 in PSUM, error ~5e-4 << 2e-2 gate.
  * scores row-tiled: head A contracts over PE rows 0-63, head B over rows
    64-127 concurrently (tile_position via base_partition).
  * DMA queues move ~114GB/s each and have ~6us start latency, so x is
    repacked host-side into [128, J, chunk, 512] J-column slices (8KB
    partition lines, one transfer per J) and phase A is J-sliced: the
    first attention block starts as soon as wq/wk/wv + x(J=0) land.
  * attention pipeline is exp-paced (scalar (N+352)/1.2); each block's
    normalize (sums->bcast->recip->mul) is emitted DEFERRED, inside the
    next block after its u=1, so it never queues ahead of the next exps;
    AV lags scores by 2 steps (pend deque) to cover the pa-bank reuse.
  * y out packed [128, 8, 2048] fp16 (host unpacks/sums), every transfer
    split in half across the sync and gpsimd queues.
"""

import numpy as np

B, T, C, H, HS = 2, 2048, 1024, 16, 64
NCORES = 8
HPC = 4            # heads per core
NKC = C // 128     # contraction chunks (8)
NJ = T // 512      # tq chunks (4)
NTS = T // 128     # ts chunks (16)

_NC_CACHE = {}


def _build_nc():
    if "nc" in _NC_CACHE:
        return _NC_CACHE["nc"]
    from contextlib import ExitStack
    import concourse.bass as bass
    from concourse import bacc, tile, mybir

    f32 = mybir.dt.float32
    f16 = mybir.dt.float16
    EXP = mybir.ActivationFunctionType.Exp

    nc = bacc.Bacc("TRN2", target_bir_lowering=False, debug=False,
                   enable_asserts=False, num_devices=NCORES)

    # prepacked inputs (per-partition-contiguous lines: x 8KB, weights 4KB)
    xt_d = nc.dram_tensor("xt_p", (128, NJ, NKC, 512), f16, kind="ExternalInput").ap()
    wq_d = nc.dram_tensor("wq_p", (128, NKC * 256), f16, kind="ExternalInput").ap()
    wk_d = nc.dram_tensor("wk_p", (128, NKC * 256), f16, kind="ExternalInput").ap()
    wv_d = nc.dram_tensor("wv_p", (128, NKC * 256), f16, kind="ExternalInput").ap()
    wp_d = nc.dram_tensor("wp_p", (128, 2 * C), f16, kind="ExternalInput").ap()
    # packed output: [p, mp, 1024*h + c] = y[128*(2*mp+h) + p, c]
    y_d = nc.dram_tensor("y", (128, 8, 2048), f16, kind="ExternalOutput").ap()

    scale = float(1.0 / np.sqrt(HS))

    with tile.TileContext(nc) as tc, ExitStack() as ctx:
        persist = ctx.enter_context(tc.tile_pool(name="persist", bufs=1))
        work = ctx.enter_context(tc.tile_pool(name="work", bufs=2))
        small = ctx.enter_context(tc.tile_pool(name="small", bufs=2))
        outp = ctx.enter_context(tc.tile_pool(name="outp", bufs=2))
        psS = ctx.enter_context(tc.tile_pool(name="psS", bufs=1, space="PSUM"))
        psatt = ctx.enter_context(tc.tile_pool(name="psatt", bufs=1, space="PSUM"))
        psaux = ctx.enter_context(tc.tile_pool(name="psaux", bufs=2, space="PSUM"))

        # ---- persistent SBUF tensors (fp16 matmul operands) ----
        xtJ = persist.tile([128, NJ, NKC, 512], f16, tag="xtJ")
        wq_sb = persist.tile([128, NKC, 256], f16, tag="wq")
        wk_sb = persist.tile([128, NKC, 256], f16, tag="wk")
        wv_sb = persist.tile([128, NKC, 256], f16, tag="wv")
        wp_sb = persist.tile([128, 2, C], f16, tag="wp")
        qT = [persist.tile([128, T], f16, tag=f"qT{p}", name=f"qT{p}") for p in range(2)]
        kT = [persist.tile([128, T], f16, tag=f"kT{p}", name=f"kT{p}") for p in range(2)]
        # v: per ts-chunk t and head h, [:, t, h, 0:64] = v dims, [..., 64] = 1.0
        vt = persist.tile([128, NTS, HPC, 65], f16, tag="vt")
        attnT = [persist.tile([128, T], f16, tag=f"attnT{p}", name=f"attnT{p}") for p in range(2)]

        # ---- loads: consumption order; sync/gpsimd/scalar queues ----
        wqf = wq_sb.rearrange("p k n -> p (k n)")
        wkf = wk_sb.rearrange("p k n -> p (k n)")
        wvf = wv_sb.rearrange("p k n -> p (k n)")
        wpf = wp_sb.rearrange("p k n -> p (k n)")
        # xJ0 split across sync+gpsimd so the first chain starts ~6us sooner
        nc.sync.dma_start(out=xtJ[:, 0, 0:4, :], in_=xt_d[:, 0, 0:4, :])
        nc.gpsimd.dma_start(out=xtJ[:, 0, 4:8, :], in_=xt_d[:, 0, 4:8, :])
        nc.scalar.dma_start(out=wqf, in_=wq_d)
        nc.gpsimd.dma_start(out=wvf, in_=wv_d)
        nc.scalar.dma_start(out=wkf, in_=wk_d)
        nc.sync.dma_start(out=xtJ[:, 1, :, :], in_=xt_d[:, 1, :, :])
        nc.gpsimd.dma_start(out=xtJ[:, 2, :, :], in_=xt_d[:, 2, :, :])
        nc.scalar.dma_start(out=xtJ[:, 3, :, :], in_=xt_d[:, 3, :, :])
        nc.sync.dma_start(out=wpf, in_=wp_d)

        # exp table preload (after the load triggers so those fire first)
        dummy = small.tile([1, 2], f32, tag="dummy")
        nc.vector.memset(dummy, 0.0)
        nc.scalar.activation(out=dummy[0:1, 1:2], in_=dummy[0:1, 0:1], func=EXP)

        nc.vector.memset(vt[:, :, :, 64:65], 1.0)

        # ---------- emission helpers ----------
        filler = []     # queue of closures emitting independent PE work
        pulled = [0]

        def pull(n):
            for _ in range(n):
                if filler:
                    filler.pop(0)()
                    pulled[0] += 1

        def qk_chain_units(pair, dst, w_sb, J, name):
            # split one 8-matmul accumulation chain into 4 filler units
            ps = psaux.tile([128, 512], f32, tag="aux", name=name)

            def unit(c0):
                def f():
                    for c in (c0, c0 + 1):
                        nc.tensor.matmul(
                            ps,
                            lhsT=w_sb[:, c, 128 * pair:128 * pair + 128],
                            rhs=xtJ[:, J, c, :],
                            start=(c == 0), stop=(c == NKC - 1))
                    if c0 == NKC - 2:
                        nc.vector.tensor_copy(
                            out=dst[:, 512 * J:512 * J + 512], in_=ps)
                return f
            return [unit(c0) for c0 in range(0, NKC, 2)]

        def qk_chain(pair, dst, w_sb, J, name):
            for u in qk_chain_units(pair, dst, w_sb, J, name):
                u()

        def v_chain(t):
            ps = psaux.tile([128, 512], f32, tag="aux", name=f"v_{t}")
            Jb, r = divmod(t, 4)
            for c in range(NKC):
                nc.tensor.matmul(
                    ps[:, 0:256],
                    lhsT=xtJ[:, Jb, c, 128 * r:128 * r + 128],
                    rhs=wv_sb[:, c, :],
                    start=(c == 0), stop=(c == NKC - 1))
            nc.vector.tensor_copy(out=vt[:, t, :, 0:64], in_=ps[:, 0:256])

        yo2 = {}

        def proj_tile(m, n):
            # proj for row-block m, col-half n; packed into [128,2048] pair
            # tiles (mp = m//2); each DMAs out as two halves on both queues.
            mp, h = divmod(m, 2)
            py_ = psaux.tile([128, 512], f32, tag="aux", name=f"y_{m}_{n}")
            for pair in range(2):
                nc.tensor.matmul(
                    py_,
                    lhsT=attnT[pair][:, 128 * m:128 * m + 128],
                    rhs=wp_sb[:, pair, 512 * n:512 * n + 512],
                    start=(pair == 0), stop=(pair == 1))
            if mp not in yo2:
                yo2[mp] = outp.tile([128, 2048], f16, tag=f"yo{mp % 2}",
                                    name=f"yo2_{mp}")
            nc.vector.tensor_copy(
                out=yo2[mp][:, 1024 * h + 512 * n:1024 * h + 512 * n + 512],
                in_=py_)
            if h == 1 and n == 1:
                t_ = yo2.pop(mp)
                nc.sync.dma_start(out=y_d[:, mp, 0:1024], in_=t_[:, 0:1024])
                nc.gpsimd.dma_start(out=y_d[:, mp, 1024:2048], in_=t_[:, 1024:2048])

        def att_pair_block(pair, J, extra=2, pending=None, after_pending=None):
            """Attention for both heads of `pair` over tq block J.

            Per ts-chunk t: row-tiled scores (A rows 0-63, B rows 64-127,
            concurrent on HW), exp, causal mask, AV. Diagonal chunks
            (t >= 4J, d = t-4J) only touch columns [128d, 512) — the rest
            is fully masked — saving PE columns and exp time. AV lags 4
            chunks behind scores (pend deque) to cover pa-bank reuse, and
            `pending` (the previous block's normalize) is emitted after
            t=1 so it never queues ahead of this block's exps.

            Returns this block's `finalize` closure for the next block.
            """
            nch = 4 * J + 4
            bjsl = slice(512 * J, 512 * J + 512)
            pa = [psatt.tile([65, 512], f32, tag=f"pa{hh}", name=f"pa{hh}_{pair}_{J}")
                  for hh in range(2)]
            pend = []          # AV four chunks behind scores

            def flush_av(last=False):
                etA, etB, t, c0 = pend.pop(0)
                for hh, et in ((0, etA), (1, etB)):
                    nc.tensor.matmul(
                        pa[hh][:, c0:512], lhsT=vt[:, t, 2 * pair + hh, :],
                        rhs=et[:, c0:512],
                        start=(t == 0),
                        stop=(last and t == nch - 1))

            for t in range(nch):
                c0 = 128 * (t - 4 * J) if t >= 4 * J else 0
                csl = slice(c0, 512)
                jsl = slice(512 * J + c0, 512 * J + 512)
                ss = [psS.tile([128, 512], f32, tag=f"s{hh}", bufs=2,
                               name=f"ss{hh}_{pair}_{J}_{t}")
                      for hh in range(2)]
                for hh in range(2):
                    hsl = slice(64 * hh, 64 * hh + 64)
                    nc.tensor.matmul(
                        ss[hh][:, csl],
                        lhsT=kT[pair][hsl, 128 * t:128 * t + 128],
                        rhs=qT[pair][hsl, jsl],
                        start=True, stop=True)
                et = [work.tile([128, 512], f16, tag=f"et{hh}", bufs=5,
                                name=f"et{hh}_{pair}_{J}_{t}")
                      for hh in range(2)]
                for hh in range(2):
                    nc.scalar.activation(out=et[hh][:, csl], in_=ss[hh][:, csl],
                                         func=EXP, scale=scale)
                if t >= 4 * J:
                    for hh in range(2):
                        # keep el iff (local col) f >= p
                        nc.gpsimd.affine_select(
                            out=et[hh][:, csl], in_=et[hh][:, csl],
                            compare_op=mybir.AluOpType.is_ge,
                            fill=0.0, base=0,
                            pattern=[[1, 512 - c0]], channel_multiplier=-1)
                pend.append((et[0], et[1], t, c0))
                if len(pend) > 4:
                    flush_av()
                if t % 2 == 1:
                    pull(extra)
                if t == min(1, nch - 1):
                    if pending is not None:
                        pending()
                    if after_pending:
                        filler.extend(after_pending)
            while pend:
                flush_av(last=len(pend) == 1)

            def finalize():
                sums, bsums, recip = [], [], []
                for hh in range(2):
                    s_ = small.tile([1, 512], f32, tag=f"sums{hh}",
                                    name=f"sums{hh}_{pair}_{J}")
                    nc.scalar.copy(out=s_, in_=pa[hh][64:65, :])
                    sums.append(s_)
                for hh in range(2):
                    b_ = small.tile([64, 512], f32, tag=f"bsums{hh}",
                                    name=f"bsums{hh}_{pair}_{J}")
                    nc.gpsimd.partition_broadcast(b_, sums[hh])
                    bsums.append(b_)
                for hh in range(2):
                    r_ = small.tile([64, 512], f32, tag=f"recip{hh}",
                                    name=f"recip{hh}_{pair}_{J}")
                    nc.vector.reciprocal_approx_fast(out=r_, in_=bsums[hh])
                    recip.append(r_)
                nc.vector.tensor_mul(attnT[pair][0:64, bjsl], pa[0][0:64, :], recip[0])
                tmp = small.tile([64, 512], f16, tag="tmp")
                nc.vector.tensor_mul(tmp, pa[1][0:64, :], recip[1])
                nc.sync.dma_start(out=attnT[pair][64:128, bjsl], in_=tmp)
            return finalize

        # ---------- J-sliced schedule: attention starts once x(J=0) lands ----
        qk_chain(0, qT[0], wq_sb, 0, "q0_0")
        qk_chain(0, kT[0], wk_sb, 0, "k0_0")
        for t in range(4):
            v_chain(t)

        # global ordered filler list: each block J's hard prerequisite is
        # q(J) fully emitted before its first scores; k(J)/v(J-range) are
        # consumed mid-block and ride the in-block pulls. `need[key]` is the
        # prefix of the list that must be emitted before that block starts.
        added = [0]
        need = {}

        def add_units(units):
            filler.extend(units)
            added[0] += len(units)

        for J in (1, 2, 3):
            add_units(qk_chain_units(0, qT[0], wq_sb, J, f"q0_{J}"))
            if J == 1:
                need[(0, 1)] = added[0]
            add_units(qk_chain_units(0, kT[0], wk_sb, J, f"k0_{J}"))
            add_units([(lambda t=t: v_chain(t)) for t in range(4 * J, 4 * J + 4)])
            if J == 2:
                need[(0, 2)] = added[0] - 8
            if J == 3:
                need[(0, 3)] = added[0] - 8
        for J in range(NJ):
            add_units(qk_chain_units(1, qT[1], wq_sb, J, f"q1_{J}"))
            if J > 0:   # k1(J) rides the in-block pulls; k1(0) is needed at u0
                need[(1, J)] = added[0]
            add_units(qk_chain_units(1, kT[1], wk_sb, J, f"k1_{J}"))
            if J == 0:
                need[(1, J)] = added[0]

        def ensure(key):
            pull(max(0, need[key] - pulled[0]))

        fin = att_pair_block(0, 0, extra=3)
        proj_of = {
            J: [(lambda m=m, n=n: (lambda: proj_tile(m, n)))()
                for m in range(4 * J, 4 * J + 4) for n in range(2)]
            for J in range(NJ)}
        for J in (1, 2, 3):
            ensure((0, J))
            fin = att_pair_block(0, J, extra=3 if J == 1 else 2, pending=fin)

        # ---------- pair 1 attention; gaps filled with proj ----------
        for J in range(NJ):
            ensure((1, J))
            fin = att_pair_block(1, J, extra=2 if J < 3 else 3, pending=fin,
                                 after_pending=proj_of[J - 1] if J > 0 else None)
        fin()
        for u_ in proj_of[3]:
            u_()
        pull(len(filler))

    nc.compile()
    _NC_CACHE["nc"] = nc
    return nc


def make_in_maps(x, wq, wk, wv, wproj):
    def packw(w):  # (C, N) -> [128, C//128, N] -> flat [128, -1]
        n = w.shape[1]
        return np.ascontiguousarray(
            w.reshape(-1, 128, n).transpose(1, 0, 2).reshape(128, -1)
        ).astype(np.float16)

    # x[b] (T, C) -> xT (C, T) -> [c, p, Jb, col] -> [p, Jb, c, col]
    xTs = [np.ascontiguousarray(
               x[b].T.reshape(NKC, 128, NJ, 512).transpose(1, 2, 0, 3)
           ).astype(np.float16)
           for b in range(B)]
    in_maps = []
    for core in range(NCORES):
        b, g = divmod(core, 4)
        hs = slice(4 * g, 4 * g + 4)
        in_maps.append({
            "xt_p": xTs[b],
            "wq_p": packw(wq[hs].transpose(1, 0, 2).reshape(C, HPC * HS)),
            "wk_p": packw(wk[hs].transpose(1, 0, 2).reshape(C, HPC * HS)),
            "wv_p": packw(wv[hs].transpose(1, 0, 2).reshape(C, HPC * HS)),
            "wp_p": packw(wproj[4 * g * HS:(4 * g + 4) * HS, :]),
        })
    return in_maps


def _assemble(results, bproj):
    y = np.zeros((B, T, C), dtype=np.float32)
    for core in range(NCORES):
        yp = results[core]["y"].astype(np.float32)        # [128, 8, 2048]
        yp = yp.reshape(128, 8, 2, 1024).transpose(1, 2, 0, 3).reshape(T, C)
        y[core // 4] += yp
    y += bproj.astype(np.float32)[None, None, :]
    return y


def _is_causal(attention_mask):
    tril = np.tril(np.ones((T, T), dtype=bool))
    return all(np.array_equal(attention_mask[b], tril) for b in range(B))


def _numpy_fallback(x, attention_mask, wq, wk, wv, wproj, bproj):
    x64 = x.astype(np.float32)
    q = np.einsum('btc,hcd->bhtd', x64, wq)
    k = np.einsum('btc,hcd->bhtd', x64, wk)
    v = np.einsum('btc,hcd->bhtd', x64, wv)
    wei = np.einsum('bhtd,bhsd->bhts', q, k) / np.sqrt(np.float32(HS))
    wei = np.where(attention_mask[:, None, :, :], wei, -np.inf)
    wei = wei - wei.max(axis=-1, keepdims=True)
    wei = np.exp(wei)
    wei = wei / wei.sum(axis=-1, keepdims=True)
    out = np.einsum('bhts,bhsd->bhtd', wei, v)
    out = out.transpose(0, 2, 1, 3).reshape(B, T, H * HS)
    return (out @ wproj + bproj).astype(np.float32)


def _install_ntff_hook():
    """Recreate the antenv.axon_hooks shim so trace=True works under axon."""
    import sys, types
    try:
        from antenv.axon_hooks import get_axon_ntff_profile_hook  # noqa
        return
    except ImportError:
        pass
    import antenv
    mod = types.ModuleType("antenv.axon_hooks")
    holder = [None]
    mod.set_axon_ntff_profile_hook = lambda h: holder.__setitem__(0, h)
    mod.get_axon_ntff_profile_hook = lambda: holder[0]
    sys.modules["antenv.axon_hooks"] = mod
    antenv.axon_hooks = mod
    if "/root/.axon_site" not in sys.path:
        sys.path.insert(0, "/root/.axon_site")
    from trn_agent_boot.trn_boot import _ntff_profile_via_ctypes
    mod.set_axon_ntff_profile_hook(_ntff_profile_via_ctypes("/opt/axon/libaxon_pjrt.so"))


def kernel(x, attention_mask, wq, wk, wv, wproj, bproj, _trace=False):
    x = np.asarray(x); attention_mask = np.asarray(attention_mask)
    wq = np.asarray(wq); wk = np.asarray(wk); wv = np.asarray(wv)
    wproj = np.asarray(wproj); bproj = np.asarray(bproj)

    if not _is_causal(attention_mask):
        return _numpy_fallback(x, attention_mask, wq, wk, wv, wproj, bproj)

    from concourse import bass_utils
    if _trace:
        _install_ntff_hook()
        bass_utils.upload_artifacts = lambda d: d
    nc = _build_nc()
    in_maps = make_in_maps(x, wq, wk, wv, wproj)
    res = bass_utils.run_bass_kernel_spmd(
        nc, in_maps, core_ids=list(range(NCORES)), trace=_trace)
    out = _assemble(res.results, bproj)
    if _trace:
        return out, res
    return out
